# revision 22
# baseline (speedup 1.0000x reference)
"""MultiHeadGraphAttention kernel for 8 Trainium2 NeuronCores.

Sharding (2D): 4 src-quarters x 2 dst-halves. Device (q, half) owns edges
with src in quarter q (12544 nodes = 98 blocks of 128) and dst in half
(25024 rows). x is uploaded bf16 as 8 disjoint shards and AllGather'd
on-device into each device's half-table; edges gather x rows via the GPSIMD
dma_gather custom op (int16 indices fit the half-table).

Per 128-edge tile (edges sorted by src within a 128-node block):
  oh[j,i] = (seg_rel[j] == i)                   (one DVE is_equal)
  y[j,(h,f)] = ee[h,j] * xg[j,f]                (broadcast DVE tensor_tensor)
  PSUM_A[i,(h,f)] += oh.T @ y                   (PE matmul, bf16)
  PSUM_R[i,h]     += oh.T @ ee                  (PE matmul, rowsums)
so each device produces PARTIAL per-head aggregates [h,b,i,f] AND rowsums
for its node quarter. The dst-half pairs are combined on-device: rowsums
via a pair AllReduce (downloaded, tiny), aggregates via a pair
ReduceScatter that head-splits [4,98,128,128] -> [2,98,128,128], so each
device downloads a unique fp16 slice that maps to the final [h,n,f] layout
with no host transpose. Host only casts + multiplies by w / rowsum, with
per-shard normalization overlapped with the (bandwidth-bound) fetch.

Edge scores ssum[e,h] = s_src[src_e,h] + s_dst[dst_e,h] are precomputed on
host (s = x @ (w*a) is a tiny [N,4] projection; scores are O(0.3) so int8
at scale 1/128 is plenty), shipped int8, and the device computes
ee = exp(-leaky_relu(s)) in batched DVE/ACT ops. Padding slots carry
seg = -1, which makes their one-hot row all-zero (no contribution to
aggregates or rowsums), so their score encoding is irrelevant.

All per-call jit state is cached module-side: the bass program, the
shard_map-jitted executable, and an on-device zeros generator for the
donated output buffers (avoids re-tracing and avoids uploading zero
buffers over the axon tunnel, which dominated wall time).
"""

import sys

sys.path.insert(0, "/opt/trn_rl_repo")

import concurrent.futures as _cf

import ml_dtypes
import numpy as np
import jax
import jax.numpy as jnp
from jax.sharding import Mesh, NamedSharding, PartitionSpec

import concourse.bass as bass  # noqa: F401  (keeps bass registered)
import concourse.tile as tile
from concourse import bacc, bass2jax, mybir
from concourse.library_config import mlp

N_NODES = 50000
H = 4
F = 128
P = 128
NCORES = 8
NQ = 4                      # src quarters
B_PER_DEV = 98              # node blocks per quarter (98*128 = 12544)
NODES_Q = B_PER_DEV * P     # 12544
HALF = 25024                # dst half-table rows (2*25024 = 50048 >= 50000)
XSH = HALF // 4             # x rows uploaded per core (AllGather x4 -> half)
NGRP = NCORES * B_PER_DEV   # 784 (dev, block) groups
SSCALE = 128.0              # int8 score quantization: s_int = round(s*128)

_last_results = None  # test.py introspection
_runner_cache = {}
_mesh = None


def _get_mesh():
    global _mesh
    if _mesh is None:
        _mesh = Mesh(np.asarray(jax.devices()[:NCORES]), ("core",))
    return _mesh


def _build_program(t_pb: int):
    """SPMD program, identical on all 8 cores; t_pb = edge tiles per block."""
    f32 = mybir.dt.float32
    bf16 = mybir.dt.bfloat16
    f16 = mybir.dt.float16
    i16 = mybir.dt.int16
    i8 = mybir.dt.int8
    T = B_PER_DEV * t_pb

    nc = bacc.Bacc("TRN2", target_bir_lowering=False, debug=False,
                   num_devices=NCORES)

    xshard = nc.dram_tensor("xshard", [XSH, F], bf16, kind="ExternalInput").ap()
    idxw16 = nc.dram_tensor("idxw16", [16, T * 8], i16, kind="ExternalInput").ap()
    segt = nc.dram_tensor("segt", [P, T], i8, kind="ExternalInput").ap()
    ssum = nc.dram_tensor("ssum", [P, 4 * T], i8, kind="ExternalInput").ap()
    iota = nc.dram_tensor("iota", [P, P], bf16, kind="ExternalInput").ap()
    xshb = nc.dram_tensor("xshb", [XSH, F], bf16, kind="Internal").ap()
    xtab = nc.dram_tensor("xtab", [HALF, F], bf16, kind="Internal").ap()
    aggf = nc.dram_tensor("aggf", [H, B_PER_DEV, P, F], f16,
                          kind="Internal").ap()
    rsf = nc.dram_tensor("rsf", [H, B_PER_DEV, P], f16, kind="Internal").ap()
    aggb = nc.dram_tensor("aggb", [H // 2, B_PER_DEV, P, F], f16,
                          kind="Internal").ap()
    rsh = nc.dram_tensor("rsh", [H // 2, B_PER_DEV, P], f16,
                         kind="Internal").ap()
    aggq = nc.dram_tensor("aggq", [H // 2, B_PER_DEV, P, F], i8,
                          kind="ExternalOutput").ap()
    sclo = nc.dram_tensor("sclo", [B_PER_DEV, P, H // 2], f16,
                          kind="ExternalOutput").ap()

    with tile.TileContext(nc) as tc:
        with (
            tc.tile_pool(name="const", bufs=1) as cpool,
            tc.tile_pool(name="gath", bufs=2) as gpool,
            tc.tile_pool(name="ework", bufs=3) as epool,
            tc.tile_pool(name="mwork", bufs=4) as mpool,
            tc.tile_pool(name="fin", bufs=2) as fpool,
            tc.tile_pool(name="psum", bufs=2, space="PSUM") as pspool,
        ):
            nc.gpsimd.load_library(mlp)

            # x AllGather: 4 shards per dst-half -> this device's half table
            # (collectives cannot touch IO tensors; bounce through Internal)
            nc.sync.dma_start(xshb[:], xshard[:])
            nc.gpsimd.collective_compute(
                "AllGather", mybir.AluOpType.bypass,
                replica_groups=[[0, 2, 4, 6], [1, 3, 5, 7]],
                ins=[xshb[:]], outs=[xtab[:]],
            )

            iota_sb = cpool.tile([P, P], bf16)
            nc.sync.dma_start(iota_sb[:], iota[:, :])

            # SBUF-resident per-edge metadata, loaded once.
            idx_sb = cpool.tile([P, T * 8], i16)
            nc.sync.dma_start(idx_sb[0:16, :], idxw16[:, :])
            nc.sync.dma_start(idx_sb[16:32, :], idx_sb[0:16, :])
            nc.sync.dma_start(idx_sb[32:64, :], idx_sb[0:32, :])
            nc.sync.dma_start(idx_sb[64:128, :], idx_sb[0:64, :])
            seg_sb = cpool.tile([P, T], i8)
            nc.sync.dma_start(seg_sb[:], segt[:, :])
            seg_f = cpool.tile([P, T], f32)
            nc.scalar.copy(seg_f[:], seg_sb[:])
            ssum_sb = cpool.tile([P, 4 * T], i8)
            nc.sync.dma_start(ssum_sb[:], ssum[:, :])
            # one upfront int8 -> f32 dequant for the whole score table
            ssum_f = cpool.tile([P, 4 * T], f32)
            nc.scalar.activation(ssum_f[:], ssum_sb[:],
                                 mybir.ActivationFunctionType.Copy,
                                 bias=0.0, scale=1.0 / SSCALE)

            for b in range(B_PER_DEV):
                sl4 = slice(4 * t_pb * b, 4 * t_pb * (b + 1))
                # ee = exp(-leaky_relu(ssum)); leaky = max(x, 0.2x)
                t0 = epool.tile([P, 4 * t_pb], f32, tag="t0")
                nc.vector.tensor_scalar(out=t0[:], in0=ssum_f[:, sl4],
                                        scalar1=0.2, scalar2=None,
                                        op0=mybir.AluOpType.mult)
                t1 = epool.tile([P, 4 * t_pb], f32, tag="t1")
                nc.vector.tensor_tensor(out=t1[:], in0=ssum_f[:, sl4],
                                        in1=t0[:], op=mybir.AluOpType.max)
                ee_b = epool.tile([P, 4 * t_pb], bf16, tag="eb")
                nc.scalar.activation(ee_b[:], t1[:],
                                     mybir.ActivationFunctionType.Exp,
                                     bias=0.0, scale=-1.0)

                # gather all of the block's x rows in one dma_gather
                xg = gpool.tile([P, t_pb * F], bf16, tag="xg")
                nc.gpsimd.dma_gather(
                    out_ap=xg[:].rearrange("p (k f) -> p k f", k=t_pb),
                    in_ap=xtab[:],
                    idxs_ap=idx_sb[:, 8 * t_pb * b:8 * t_pb * (b + 1)],
                    num_idxs=t_pb * P,
                    num_idxs_reg=t_pb * P,
                    elem_size=F,
                    single_packet=False,
                )

                agg_ps = pspool.tile([P, H * P], f32, tag="agg")
                rs_ps = pspool.tile([P, H], f32, tag="rs")
                for t in range(t_pb):
                    oh = mpool.tile([P, P], bf16, tag="oh")
                    nc.vector.tensor_scalar(
                        out=oh[:], in0=iota_sb[:],
                        scalar1=seg_f[:, b * t_pb + t:b * t_pb + t + 1],
                        scalar2=None, op0=mybir.AluOpType.is_equal)
                    y = mpool.tile([P, H * P], bf16, tag="y")
                    xgt = xg[:, t * F:(t + 1) * F]
                    eet = ee_b[:, 4 * t:4 * t + 4]
                    nc.vector.tensor_tensor(
                        out=y[:].rearrange("p (h f) -> p h f", h=H),
                        in0=xgt.rearrange("p (o f) -> p o f", o=1)
                            .broadcast_to([P, H, F]),
                        in1=eet.rearrange("p (h o) -> p h o", o=1)
                            .broadcast_to([P, H, F]),
                        op=mybir.AluOpType.mult)
                    nc.tensor.matmul(out=agg_ps[:], lhsT=oh[:], rhs=y[:],
                                     start=(t == 0), stop=(t == t_pb - 1))
                    nc.tensor.matmul(out=rs_ps[:], lhsT=oh[:], rhs=eet,
                                     start=(t == 0), stop=(t == t_pb - 1))

                osb = fpool.tile([P, H * P], f16, tag="osb")
                nc.scalar.copy(osb[:], agg_ps[:])
                rsb = fpool.tile([P, H], f16, tag="rsb")
                nc.scalar.copy(rsb[:], rs_ps[:])
                nc.sync.dma_start(
                    aggf[:, b, :, :].rearrange("h p f -> p h f"),
                    osb[:].rearrange("p (h f) -> p h f", h=H))
                nc.sync.dma_start(rsf[:, b, :].rearrange("h p -> p h"),
                                  rsb[:])

            # pair-combine the dst halves on device: aggregates head-split
            # via ReduceScatter ([4,98,128,128] -> [2,98,128,128]), rowsums
            # AllReduce'd (tiny, host divides)
            nc.gpsimd.collective_compute(
                "ReduceScatter", mybir.AluOpType.add,
                replica_groups=[[0, 1], [2, 3], [4, 5], [6, 7]],
                ins=[aggf[:]], outs=[aggb[:]],
            )
            nc.gpsimd.collective_compute(
                "ReduceScatter", mybir.AluOpType.add,
                replica_groups=[[0, 1], [2, 3], [4, 5], [6, 7]],
                ins=[rsf[:]], outs=[rsh[:]],
            )

            # int8 quantization of the pair-summed aggregates with a
            # per-(node,head) amax scale: halves the (bandwidth-bound)
            # device->host fetch. The rowsum division is folded into the
            # downloaded scale: sclo = amax/(127*rowsum); host just
            # multiplies by sclo and w.
            H2 = H // 2
            for b in range(B_PER_DEV):
                ab = mpool.tile([P, H2 * F], f16, tag="qab")
                nc.sync.dma_start(
                    ab[:].rearrange("p (h f) -> p h f", h=H2),
                    aggb[:, b, :, :].rearrange("h p f -> p h f"))
                rs2 = fpool.tile([P, H2], f16, tag="qrs")
                nc.sync.dma_start(rs2[:],
                                  rsh[:, b, :].rearrange("h p -> p h"))
                amx = epool.tile([P, H2], f32, tag="qam")
                nc.vector.tensor_reduce(
                    out=amx[:], in_=ab[:].rearrange("p (h f) -> p h f", h=H2),
                    axis=mybir.AxisListType.X, op=mybir.AluOpType.max,
                    apply_absolute_value=True)
                amc = epool.tile([P, H2], f32, tag="qac")
                nc.vector.tensor_scalar(out=amc[:], in0=amx[:],
                                        scalar1=1e-20, scalar2=None,
                                        op0=mybir.AluOpType.max)
                rcp = epool.tile([P, H2], f32, tag="qrc")
                nc.vector.reciprocal(rcp[:], amc[:])
                qm = epool.tile([P, H2], f32, tag="qqm")
                nc.vector.tensor_scalar(out=qm[:], in0=rcp[:],
                                        scalar1=127.0, scalar2=None,
                                        op0=mybir.AluOpType.mult)
                qv = mpool.tile([P, H2 * F], i8, tag="qqv")
                nc.vector.tensor_tensor(
                    out=qv[:].rearrange("p (h f) -> p h f", h=H2),
                    in0=ab[:].rearrange("p (h f) -> p h f", h=H2),
                    in1=qm[:].rearrange("p (h o) -> p h o", o=1)
                        .broadcast_to([P, H2, F]),
                    op=mybir.AluOpType.mult)
                # sclo = amax/(127*rowsum)
                rsc = epool.tile([P, H2], f32, tag="qr2")
                nc.vector.tensor_scalar(out=rsc[:], in0=rs2[:],
                                        scalar1=1e-20, scalar2=None,
                                        op0=mybir.AluOpType.max)
                rrc = epool.tile([P, H2], f32, tag="qr3")
                nc.vector.reciprocal(rrc[:], rsc[:])
                fac = epool.tile([P, H2], f32, tag="qfc")
                nc.vector.tensor_tensor(out=fac[:], in0=amc[:], in1=rrc[:],
                                        op=mybir.AluOpType.mult)
                scb = fpool.tile([P, H2], f16, tag="qsc")
                nc.scalar.activation(scb[:], fac[:],
                                     mybir.ActivationFunctionType.Copy,
                                     bias=0.0, scale=1.0 / 127.0)
                nc.sync.dma_start(
                    aggq[:, b, :, :].rearrange("h p f -> p h f"),
                    qv[:].rearrange("p (h f) -> p h f", h=H2))
                nc.sync.dma_start(sclo[b], scb[:])
    nc.compile()
    return nc


class _Runner:
    __slots__ = ("nc", "sharded", "zeros", "in_names", "out_names", "n_params")


def _get_runner(t_pb: int) -> _Runner:
    r = _runner_cache.get(t_pb)
    if r is not None:
        return r
    nc = _build_program(t_pb)
    bass2jax.install_neuronx_cc_hook()
    pn = nc.partition_id_tensor.name if nc.partition_id_tensor else None
    in_names, out_names, out_avals = [], [], []
    for alloc in nc.m.functions[0].allocations:
        if not isinstance(alloc, mybir.MemoryLocationSet):
            continue
        name = alloc.memorylocations[0].name
        if alloc.kind == "ExternalInput":
            if name != pn:
                in_names.append(name)
        elif alloc.kind == "ExternalOutput":
            out_names.append(name)
            out_avals.append(jax.core.ShapedArray(
                tuple(alloc.tensor_shape), mybir.dt.np(alloc.dtype)))
    all_names = tuple(in_names + out_names + ([pn] if pn else []))
    n_params = len(in_names)
    n_outs = len(out_names)

    def _body(*args):
        operands = list(args)
        if pn is not None:
            operands.append(bass2jax.partition_id_tensor())
        return tuple(bass2jax._bass_exec_p.bind(
            *operands, out_avals=tuple(out_avals), in_names=all_names,
            out_names=tuple(out_names), lowering_input_output_aliases=(),
            sim_require_finite=True, sim_require_nnan=True, nc=nc))

    from jax.experimental.shard_map import shard_map
    mesh = _get_mesh()
    spec = PartitionSpec("core")
    sharded = jax.jit(
        shard_map(_body, mesh=mesh, in_specs=(spec,) * (n_params + n_outs),
                  out_specs=(spec,) * n_outs, check_rep=False),
        donate_argnums=tuple(range(n_params, n_params + n_outs)),
        keep_unused=True)

    sh = NamedSharding(mesh, spec)
    zshapes = [(NCORES * av.shape[0], *av.shape[1:]) for av in out_avals]
    zdtypes = [av.dtype for av in out_avals]
    zeros = jax.jit(
        lambda: tuple(jnp.zeros(s, d) for s, d in zip(zshapes, zdtypes)),
        out_shardings=(sh,) * n_outs)

    r = _Runner()
    r.nc, r.sharded, r.zeros = nc, sharded, zeros
    r.in_names, r.out_names, r.n_params = in_names, out_names, n_params
    _runner_cache[t_pb] = r
    return r


def kernel(x, w, a, edge_index):
    global _last_results
    _last_results = None
    x = np.asarray(x, dtype=np.float32)
    w = np.asarray(w, dtype=np.float32)
    a = np.asarray(a, dtype=np.float32)
    edge_index = np.asarray(edge_index)
    n = x.shape[0]

    sh = NamedSharding(_get_mesh(), PartitionSpec("core"))

    # ship x early so the upload overlaps the host-side edge preprocessing
    x_pad = np.zeros((2 * HALF, F), np.float32)
    x_pad[:n] = x
    x_bf = x_pad.astype(ml_dtypes.bfloat16)
    xg_np = np.ascontiguousarray(
        x_bf.reshape(2, 4, XSH, F).transpose(1, 0, 2, 3)).reshape(-1, F)
    xg_dev = jax.device_put(xg_np, sh)

    src = edge_index[0].astype(np.int32)
    dst = edge_index[1].astype(np.int32)
    E = src.shape[0]

    # tiny per-node projections: s = x @ (w*a_part).T per head
    c_src = (w[:, 0, :] * a[:, :F, 0]).astype(np.float32)
    c_dst = (w[:, 0, :] * a[:, F:, 0]).astype(np.float32)
    s_src = x @ c_src.T  # [N,H]
    s_dst = x @ c_dst.T

    half = (dst >= HALF).astype(np.int32)
    grp = ((src // NODES_Q) * 2 + half) * B_PER_DEV + ((src % NODES_Q) >> 7)
    # pre-sort compact encodings (gathered by `order` below)
    dst_rel = (dst - half * HALF).astype(np.int16)
    seg8 = (src & 127).astype(np.int8)
    sq8 = np.clip(np.rint((s_src[src] + s_dst[dst]) * SSCALE),
                  -127, 127).astype(np.int8)  # [E,H]

    order = np.argsort(grp.astype(np.uint16), kind="stable")
    g_s = grp[order]

    counts = np.bincount(grp, minlength=NGRP)
    t_pb = max(1, (int(counts.max()) + P - 1) // P)
    spb = t_pb * P
    T = B_PER_DEV * t_pb
    starts = np.zeros(NGRP, np.int32)
    np.cumsum(counts[:-1], out=starts[1:])
    slot = g_s * spb + (np.arange(E, dtype=np.int32) - starts[g_s])
    nslots = NGRP * spb

    dst_slots = np.zeros(nslots, np.int16)
    dst_slots[slot] = dst_rel[order]
    seg_slots = np.full(nslots, -1, np.int8)  # -1 = padding: all-zero onehot
    seg_slots[slot] = seg8[order]
    ssum_slots = np.zeros((nslots, H), np.int8)
    ssum_slots[slot] = sq8[order]

    # device layouts (global, core-major along axis 0)
    idxw_np = np.ascontiguousarray(
        dst_slots.reshape(NCORES, B_PER_DEV, spb // 16, 16)
        .transpose(0, 3, 1, 2)).reshape(NCORES * 16, B_PER_DEV * (spb // 16))
    segt_np = np.ascontiguousarray(
        seg_slots.reshape(NCORES, T, P).transpose(0, 2, 1)).reshape(
        NCORES * P, T)
    ssum_np = np.ascontiguousarray(
        ssum_slots.reshape(NCORES, T, P, H).transpose(0, 2, 1, 3)).reshape(
        NCORES * P, 4 * T)
    iota_np = np.tile(np.broadcast_to(
        np.arange(P, dtype=np.float32), (P, P)).astype(ml_dtypes.bfloat16),
        (NCORES, 1))

    runner = _get_runner(t_pb)
    zeros = runner.zeros()  # device-side, dispatched before the uploads
    in_np = {"xshard": xg_dev, "idxw16": idxw_np, "segt": segt_np,
             "ssum": ssum_np, "iota": iota_np}
    ins = [in_np[name] if isinstance(in_np[name], jax.Array)
           else jax.device_put(in_np[name], sh) for name in runner.in_names]
    outs = runner.sharded(*ins, *zeros)
    out_by_name = dict(zip(runner.out_names, outs))
    for o in outs:  # start all D2H transfers without blocking
        for s in o.addressable_shards:
            s.data.copy_to_host_async()

    # aggregates: core c=2q+r holds heads (2r, 2r+1) of quarter q in
    # [2, 98, 128, 128] = [h', n_local, f] int8 + per-(node,head) combined
    # scales (amax/127/rowsum) -> dequant+normalize fused per shard, no
    # transpose, overlapped with the bandwidth-bound fetch.
    out_full = np.empty((H, N_NODES, F), np.float32)
    aggq = out_by_name["aggq"]
    sclo = out_by_name["sclo"]
    wn = w[:, 0, :]  # [H, F]

    def _fetch_and_norm(c):
        shard = np.asarray(aggq.addressable_shards[c].data)  # [2,98,128,128]
        scl = np.asarray(sclo.addressable_shards[c].data)    # [98,128,2]
        q, rr = divmod(c, 2)
        hs = slice(2 * rr, 2 * rr + 2)
        lo = q * NODES_Q
        nn = min(NODES_Q, N_NODES - lo)
        if nn <= 0:
            return
        fac = scl.reshape(NODES_Q, 2).T[:, :nn].astype(np.float32)
        dst_v = out_full[hs, lo:lo + nn, :]
        np.multiply(shard.reshape(2, NODES_Q, F)[:, :nn],
                    fac[:, :, None], out=dst_v, casting="unsafe")
        dst_v *= wn[hs][:, None, :]

    with _cf.ThreadPoolExecutor(4) as ex:
        list(ex.map(_fetch_and_norm, range(NCORES)))

    return out_full


# revision 26
# speedup vs baseline: 1.1127x; 1.1127x over previous
"""MultiHeadGraphAttention kernel for 8 Trainium2 NeuronCores.

Sharding (2D): 4 src-quarters x 2 dst-halves. Device (q, half) owns edges
with src in quarter q (12544 nodes = 98 blocks of 128) and dst in half
(25024 rows). x is uploaded bf16 as 8 disjoint shards and AllGather'd
on-device into each device's half-table; edges gather x rows via the GPSIMD
dma_gather custom op (int16 indices fit the half-table).

Per 128-edge tile (edges sorted by src within a 128-node block):
  oh[j,i] = (seg_rel[j] == i)                   (one DVE is_equal)
  y[j,(h,f)] = ee[h,j] * xg[j,f]                (broadcast DVE tensor_tensor)
  PSUM_A[i,(h,f)] += oh.T @ y                   (PE matmul, bf16)
  PSUM_R[i,h]     += oh.T @ ee                  (PE matmul, rowsums)
so each device produces PARTIAL per-head aggregates [h,b,i,f] AND rowsums
for its node quarter. The dst-half pairs are combined on-device: rowsums
via a pair AllReduce (downloaded, tiny), aggregates via a pair
ReduceScatter that head-splits [4,98,128,128] -> [2,98,128,128], so each
device downloads a unique fp16 slice that maps to the final [h,n,f] layout
with no host transpose. Host only casts + multiplies by w / rowsum, with
per-shard normalization overlapped with the (bandwidth-bound) fetch.

Edge scores ssum[e,h] = s_src[src_e,h] + s_dst[dst_e,h] are precomputed on
host (s = x @ (w*a) is a tiny [N,4] projection; scores are O(0.3) so int8
at scale 1/128 is plenty), shipped int8, and the device computes
ee = exp(-leaky_relu(s)) in batched DVE/ACT ops. Padding slots carry
seg = -1, which makes their one-hot row all-zero (no contribution to
aggregates or rowsums), so their score encoding is irrelevant.

All per-call jit state is cached module-side: the bass program, the
shard_map-jitted executable, and an on-device zeros generator for the
donated output buffers (avoids re-tracing and avoids uploading zero
buffers over the axon tunnel, which dominated wall time).
"""

import sys

sys.path.insert(0, "/opt/trn_rl_repo")

import concurrent.futures as _cf

import ml_dtypes
import numpy as np
import jax
import jax.numpy as jnp
from jax.sharding import Mesh, NamedSharding, PartitionSpec

import concourse.bass as bass  # noqa: F401  (keeps bass registered)
import concourse.tile as tile
from concourse import bacc, bass2jax, mybir
from concourse.library_config import mlp

N_NODES = 50000
H = 4
F = 128
P = 128
NCORES = 8
NQ = 4                      # src quarters
B_PER_DEV = 98              # node blocks per quarter (98*128 = 12544)
NODES_Q = B_PER_DEV * P     # 12544
HALF = 25024                # dst half-table rows (2*25024 = 50048 >= 50000)
XSH = HALF // 4             # x rows uploaded per core (AllGather x4 -> half)
NGRP = NCORES * B_PER_DEV   # 784 (dev, block) groups
SSCALE = 128.0              # int8 score quantization: s_int = round(s*128)

_last_results = None  # test.py introspection
_runner_cache = {}
_mesh = None


def _get_mesh():
    global _mesh
    if _mesh is None:
        _mesh = Mesh(np.asarray(jax.devices()[:NCORES]), ("core",))
    return _mesh


def _build_program(t_pb: int):
    """SPMD program, identical on all 8 cores; t_pb = edge tiles per block."""
    f32 = mybir.dt.float32
    bf16 = mybir.dt.bfloat16
    f16 = mybir.dt.float16
    i16 = mybir.dt.int16
    i8 = mybir.dt.int8
    T = B_PER_DEV * t_pb

    nc = bacc.Bacc("TRN2", target_bir_lowering=False, debug=False,
                   num_devices=NCORES)

    xshard = nc.dram_tensor("xshard", [XSH, F], bf16, kind="ExternalInput").ap()
    idxw16 = nc.dram_tensor("idxw16", [16, T * 8], i16, kind="ExternalInput").ap()
    segt = nc.dram_tensor("segt", [P, T], i8, kind="ExternalInput").ap()
    ssum = nc.dram_tensor("ssum", [P, 4 * T], i8, kind="ExternalInput").ap()
    iota = nc.dram_tensor("iota", [P, P], bf16, kind="ExternalInput").ap()
    xshb = nc.dram_tensor("xshb", [XSH, F], bf16, kind="Internal").ap()
    xtab = nc.dram_tensor("xtab", [HALF, F], bf16, kind="Internal").ap()
    aggf = nc.dram_tensor("aggf", [H, B_PER_DEV, P, F], f16,
                          kind="Internal").ap()
    rsf = nc.dram_tensor("rsf", [H, B_PER_DEV, P], f16, kind="Internal").ap()
    aggb = nc.dram_tensor("aggb", [H // 2, B_PER_DEV, P, F], f16,
                          kind="Internal").ap()
    rsh = nc.dram_tensor("rsh", [H // 2, B_PER_DEV, P], f16,
                         kind="Internal").ap()
    aggq = nc.dram_tensor("aggq", [H // 2, B_PER_DEV, P, F], i8,
                          kind="ExternalOutput").ap()
    sclo = nc.dram_tensor("sclo", [B_PER_DEV, P, H // 2], f16,
                          kind="ExternalOutput").ap()

    with tile.TileContext(nc) as tc:
        with (
            tc.tile_pool(name="const", bufs=1) as cpool,
            tc.tile_pool(name="gath", bufs=2) as gpool,
            tc.tile_pool(name="ework", bufs=3) as epool,
            tc.tile_pool(name="mwork", bufs=4) as mpool,
            tc.tile_pool(name="fin", bufs=2) as fpool,
            tc.tile_pool(name="psum", bufs=2, space="PSUM") as pspool,
        ):
            nc.gpsimd.load_library(mlp)

            # x AllGather: 4 shards per dst-half -> this device's half table
            # (collectives cannot touch IO tensors; bounce through Internal)
            nc.sync.dma_start(xshb[:], xshard[:])
            nc.gpsimd.collective_compute(
                "AllGather", mybir.AluOpType.bypass,
                replica_groups=[[0, 2, 4, 6], [1, 3, 5, 7]],
                ins=[xshb[:]], outs=[xtab[:]],
            )

            iota_sb = cpool.tile([P, P], bf16)
            nc.sync.dma_start(iota_sb[:], iota[:, :])

            # SBUF-resident per-edge metadata, loaded once.
            idx_sb = cpool.tile([P, T * 8], i16)
            nc.sync.dma_start(idx_sb[0:16, :], idxw16[:, :])
            nc.sync.dma_start(idx_sb[16:32, :], idx_sb[0:16, :])
            nc.sync.dma_start(idx_sb[32:64, :], idx_sb[0:32, :])
            nc.sync.dma_start(idx_sb[64:128, :], idx_sb[0:64, :])
            seg_sb = cpool.tile([P, T], i8)
            nc.sync.dma_start(seg_sb[:], segt[:, :])
            seg_f = cpool.tile([P, T], f32)
            nc.scalar.copy(seg_f[:], seg_sb[:])
            ssum_sb = cpool.tile([P, 4 * T], i8)
            nc.sync.dma_start(ssum_sb[:], ssum[:, :])
            # one upfront int8 -> f32 dequant for the whole score table
            ssum_f = cpool.tile([P, 4 * T], f32)
            nc.scalar.activation(ssum_f[:], ssum_sb[:],
                                 mybir.ActivationFunctionType.Copy,
                                 bias=0.0, scale=1.0 / SSCALE)

            for b in range(B_PER_DEV):
                sl4 = slice(4 * t_pb * b, 4 * t_pb * (b + 1))
                # ee = exp(-leaky_relu(ssum)); leaky = max(x, 0.2x)
                t0 = epool.tile([P, 4 * t_pb], f32, tag="t0")
                nc.vector.tensor_scalar(out=t0[:], in0=ssum_f[:, sl4],
                                        scalar1=0.2, scalar2=None,
                                        op0=mybir.AluOpType.mult)
                t1 = epool.tile([P, 4 * t_pb], f32, tag="t1")
                nc.vector.tensor_tensor(out=t1[:], in0=ssum_f[:, sl4],
                                        in1=t0[:], op=mybir.AluOpType.max)
                ee_b = epool.tile([P, 4 * t_pb], bf16, tag="eb")
                nc.scalar.activation(ee_b[:], t1[:],
                                     mybir.ActivationFunctionType.Exp,
                                     bias=0.0, scale=-1.0)

                # gather all of the block's x rows in one dma_gather
                xg = gpool.tile([P, t_pb * F], bf16, tag="xg")
                nc.gpsimd.dma_gather(
                    out_ap=xg[:].rearrange("p (k f) -> p k f", k=t_pb),
                    in_ap=xtab[:],
                    idxs_ap=idx_sb[:, 8 * t_pb * b:8 * t_pb * (b + 1)],
                    num_idxs=t_pb * P,
                    num_idxs_reg=t_pb * P,
                    elem_size=F,
                    single_packet=False,
                )

                agg_ps = pspool.tile([P, H * P], f32, tag="agg")
                rs_ps = pspool.tile([P, H], f32, tag="rs")
                for t in range(t_pb):
                    oh = mpool.tile([P, P], bf16, tag="oh")
                    nc.vector.tensor_scalar(
                        out=oh[:], in0=iota_sb[:],
                        scalar1=seg_f[:, b * t_pb + t:b * t_pb + t + 1],
                        scalar2=None, op0=mybir.AluOpType.is_equal)
                    y = mpool.tile([P, H * P], bf16, tag="y")
                    xgt = xg[:, t * F:(t + 1) * F]
                    eet = ee_b[:, 4 * t:4 * t + 4]
                    nc.vector.tensor_tensor(
                        out=y[:].rearrange("p (h f) -> p h f", h=H),
                        in0=xgt.rearrange("p (o f) -> p o f", o=1)
                            .broadcast_to([P, H, F]),
                        in1=eet.rearrange("p (h o) -> p h o", o=1)
                            .broadcast_to([P, H, F]),
                        op=mybir.AluOpType.mult)
                    nc.tensor.matmul(out=agg_ps[:], lhsT=oh[:], rhs=y[:],
                                     start=(t == 0), stop=(t == t_pb - 1))
                    nc.tensor.matmul(out=rs_ps[:], lhsT=oh[:], rhs=eet,
                                     start=(t == 0), stop=(t == t_pb - 1))

                osb = fpool.tile([P, H * P], f16, tag="osb")
                nc.scalar.copy(osb[:], agg_ps[:])
                rsb = fpool.tile([P, H], f16, tag="rsb")
                nc.scalar.copy(rsb[:], rs_ps[:])
                nc.sync.dma_start(
                    aggf[:, b, :, :].rearrange("h p f -> p h f"),
                    osb[:].rearrange("p (h f) -> p h f", h=H))
                nc.sync.dma_start(rsf[:, b, :].rearrange("h p -> p h"),
                                  rsb[:])

            # pair-combine the dst halves on device: aggregates head-split
            # via ReduceScatter ([4,98,128,128] -> [2,98,128,128]), rowsums
            # AllReduce'd (tiny, host divides)
            nc.gpsimd.collective_compute(
                "ReduceScatter", mybir.AluOpType.add,
                replica_groups=[[0, 1], [2, 3], [4, 5], [6, 7]],
                ins=[aggf[:]], outs=[aggb[:]],
            )
            nc.gpsimd.collective_compute(
                "ReduceScatter", mybir.AluOpType.add,
                replica_groups=[[0, 1], [2, 3], [4, 5], [6, 7]],
                ins=[rsf[:]], outs=[rsh[:]],
            )

            # int8 quantization of the pair-summed aggregates with a
            # per-(node,head) amax scale: halves the (bandwidth-bound)
            # device->host fetch. The rowsum division is folded into the
            # downloaded scale: sclo = amax/(127*rowsum); host just
            # multiplies by sclo and w.
            H2 = H // 2
            for b in range(B_PER_DEV):
                ab = mpool.tile([P, H2 * F], f16, tag="qab")
                nc.sync.dma_start(
                    ab[:].rearrange("p (h f) -> p h f", h=H2),
                    aggb[:, b, :, :].rearrange("h p f -> p h f"))
                rs2 = fpool.tile([P, H2], f16, tag="qrs")
                nc.sync.dma_start(rs2[:],
                                  rsh[:, b, :].rearrange("h p -> p h"))
                amx = epool.tile([P, H2], f32, tag="qam")
                nc.vector.tensor_reduce(
                    out=amx[:], in_=ab[:].rearrange("p (h f) -> p h f", h=H2),
                    axis=mybir.AxisListType.X, op=mybir.AluOpType.max,
                    apply_absolute_value=True)
                amc = epool.tile([P, H2], f32, tag="qac")
                nc.vector.tensor_scalar(out=amc[:], in0=amx[:],
                                        scalar1=1e-20, scalar2=None,
                                        op0=mybir.AluOpType.max)
                rcp = epool.tile([P, H2], f32, tag="qrc")
                nc.vector.reciprocal(rcp[:], amc[:])
                qm = epool.tile([P, H2], f32, tag="qqm")
                nc.vector.tensor_scalar(out=qm[:], in0=rcp[:],
                                        scalar1=127.0, scalar2=None,
                                        op0=mybir.AluOpType.mult)
                qv = mpool.tile([P, H2 * F], i8, tag="qqv")
                nc.vector.tensor_tensor(
                    out=qv[:].rearrange("p (h f) -> p h f", h=H2),
                    in0=ab[:].rearrange("p (h f) -> p h f", h=H2),
                    in1=qm[:].rearrange("p (h o) -> p h o", o=1)
                        .broadcast_to([P, H2, F]),
                    op=mybir.AluOpType.mult)
                # sclo = amax/(127*rowsum)
                rsc = epool.tile([P, H2], f32, tag="qr2")
                nc.vector.tensor_scalar(out=rsc[:], in0=rs2[:],
                                        scalar1=1e-20, scalar2=None,
                                        op0=mybir.AluOpType.max)
                rrc = epool.tile([P, H2], f32, tag="qr3")
                nc.vector.reciprocal(rrc[:], rsc[:])
                fac = epool.tile([P, H2], f32, tag="qfc")
                nc.vector.tensor_tensor(out=fac[:], in0=amc[:], in1=rrc[:],
                                        op=mybir.AluOpType.mult)
                scb = fpool.tile([P, H2], f16, tag="qsc")
                nc.scalar.activation(scb[:], fac[:],
                                     mybir.ActivationFunctionType.Copy,
                                     bias=0.0, scale=1.0 / 127.0)
                nc.sync.dma_start(
                    aggq[:, b, :, :].rearrange("h p f -> p h f"),
                    qv[:].rearrange("p (h f) -> p h f", h=H2))
                nc.sync.dma_start(sclo[b], scb[:])
    nc.compile()
    return nc


class _Runner:
    __slots__ = ("nc", "sharded", "zeros", "in_names", "out_names",
                 "n_params", "next_zeros")


def _get_runner(t_pb: int) -> _Runner:
    r = _runner_cache.get(t_pb)
    if r is not None:
        return r
    nc = _build_program(t_pb)
    bass2jax.install_neuronx_cc_hook()
    pn = nc.partition_id_tensor.name if nc.partition_id_tensor else None
    in_names, out_names, out_avals = [], [], []
    for alloc in nc.m.functions[0].allocations:
        if not isinstance(alloc, mybir.MemoryLocationSet):
            continue
        name = alloc.memorylocations[0].name
        if alloc.kind == "ExternalInput":
            if name != pn:
                in_names.append(name)
        elif alloc.kind == "ExternalOutput":
            out_names.append(name)
            out_avals.append(jax.core.ShapedArray(
                tuple(alloc.tensor_shape), mybir.dt.np(alloc.dtype)))
    all_names = tuple(in_names + out_names + ([pn] if pn else []))
    n_params = len(in_names)
    n_outs = len(out_names)

    def _body(*args):
        operands = list(args)
        if pn is not None:
            operands.append(bass2jax.partition_id_tensor())
        return tuple(bass2jax._bass_exec_p.bind(
            *operands, out_avals=tuple(out_avals), in_names=all_names,
            out_names=tuple(out_names), lowering_input_output_aliases=(),
            sim_require_finite=True, sim_require_nnan=True, nc=nc))

    from jax.experimental.shard_map import shard_map
    mesh = _get_mesh()
    spec = PartitionSpec("core")
    sharded = jax.jit(
        shard_map(_body, mesh=mesh, in_specs=(spec,) * (n_params + n_outs),
                  out_specs=(spec,) * n_outs, check_rep=False),
        donate_argnums=tuple(range(n_params, n_params + n_outs)),
        keep_unused=True)

    sh = NamedSharding(mesh, spec)
    zshapes = [(NCORES * av.shape[0], *av.shape[1:]) for av in out_avals]
    zdtypes = [av.dtype for av in out_avals]
    zeros = jax.jit(
        lambda: tuple(jnp.zeros(s, d) for s, d in zip(zshapes, zdtypes)),
        out_shardings=(sh,) * n_outs)

    r = _Runner()
    r.nc, r.sharded, r.zeros = nc, sharded, zeros
    r.in_names, r.out_names, r.n_params = in_names, out_names, n_params
    r.next_zeros = None
    _runner_cache[t_pb] = r
    return r


def kernel(x, w, a, edge_index):
    global _last_results
    _last_results = None
    x = np.asarray(x, dtype=np.float32)
    w = np.asarray(w, dtype=np.float32)
    a = np.asarray(a, dtype=np.float32)
    edge_index = np.asarray(edge_index)
    n = x.shape[0]

    sh = NamedSharding(_get_mesh(), PartitionSpec("core"))
    pool = _cf.ThreadPoolExecutor(4)

    def _x_branch():
        # ship x early so the upload overlaps the edge preprocessing
        x_pad = np.zeros((2 * HALF, F), np.float32)
        x_pad[:n] = x
        x_bf = x_pad.astype(ml_dtypes.bfloat16)
        xg_np = np.ascontiguousarray(
            x_bf.reshape(2, 4, XSH, F).transpose(1, 0, 2, 3)).reshape(-1, F)
        return jax.device_put(xg_np, sh)

    fut_x = pool.submit(_x_branch)

    src = edge_index[0].astype(np.int32)
    dst = edge_index[1].astype(np.int32)
    E = src.shape[0]
    half = (dst >= HALF).astype(np.int32)

    def _scores():
        # tiny per-node projections: s = x @ (w*a_part).T per head
        c_src = (w[:, 0, :] * a[:, :F, 0]).astype(np.float32)
        c_dst = (w[:, 0, :] * a[:, F:, 0]).astype(np.float32)
        s_src = x @ c_src.T  # [N,H]
        s_dst = x @ c_dst.T
        return np.clip(np.rint((s_src[src] + s_dst[dst]) * SSCALE),
                       -127, 127).astype(np.int8)  # [E,H]

    def _encodings():
        return (dst - half * HALF).astype(np.int16), (src & 127).astype(np.int8)

    fut_sq = pool.submit(_scores)
    fut_enc = pool.submit(_encodings)

    grp = ((src // NODES_Q) * 2 + half) * B_PER_DEV + ((src % NODES_Q) >> 7)
    order = np.argsort(grp.astype(np.uint16), kind="stable")
    g_s = grp[order]

    counts = np.bincount(grp, minlength=NGRP)
    t_pb = max(1, (int(counts.max()) + P - 1) // P)
    spb = t_pb * P
    T = B_PER_DEV * t_pb
    starts = np.zeros(NGRP, np.int32)
    np.cumsum(counts[:-1], out=starts[1:])
    slot = g_s * spb + (np.arange(E, dtype=np.int32) - starts[g_s])
    nslots = NGRP * spb

    runner = _get_runner(t_pb)
    zeros = runner.next_zeros if runner.next_zeros is not None \
        else runner.zeros()
    runner.next_zeros = None

    def _build_idxw():
        dst_rel, _ = fut_enc.result()
        dst_slots = np.zeros(nslots, np.int16)
        dst_slots[slot] = dst_rel[order]
        idxw_np = np.ascontiguousarray(
            dst_slots.reshape(NCORES, B_PER_DEV, spb // 16, 16)
            .transpose(0, 3, 1, 2)).reshape(NCORES * 16,
                                            B_PER_DEV * (spb // 16))
        return jax.device_put(idxw_np, sh)

    def _build_segt():
        _, seg8 = fut_enc.result()
        seg_slots = np.full(nslots, -1, np.int8)  # -1 pad: all-zero onehot
        seg_slots[slot] = seg8[order]
        segt_np = np.ascontiguousarray(
            seg_slots.reshape(NCORES, T, P).transpose(0, 2, 1)).reshape(
            NCORES * P, T)
        return jax.device_put(segt_np, sh)

    def _build_ssum():
        ssum_slots = np.zeros((nslots, H), np.int8)
        ssum_slots[slot] = fut_sq.result()[order]
        ssum_np = np.ascontiguousarray(
            ssum_slots.reshape(NCORES, T, P, H).transpose(0, 2, 1, 3)
        ).reshape(NCORES * P, 4 * T)
        return jax.device_put(ssum_np, sh)

    fut_ssum = pool.submit(_build_ssum)
    fut_idxw = pool.submit(_build_idxw)
    fut_segt = pool.submit(_build_segt)
    iota_np = np.tile(np.broadcast_to(
        np.arange(P, dtype=np.float32), (P, P)).astype(ml_dtypes.bfloat16),
        (NCORES, 1))
    in_dev = {"iota": jax.device_put(iota_np, sh), "xshard": fut_x.result(),
              "idxw16": fut_idxw.result(), "segt": fut_segt.result(),
              "ssum": fut_ssum.result()}
    ins = [in_dev[name] for name in runner.in_names]
    outs = runner.sharded(*ins, *zeros)
    out_by_name = dict(zip(runner.out_names, outs))
    for o in outs:  # start all D2H transfers without blocking
        for s in o.addressable_shards:
            s.data.copy_to_host_async()

    # aggregates: core c=2q+r holds heads (2r, 2r+1) of quarter q in
    # [2, 98, 128, 128] = [h', n_local, f] int8 + per-(node,head) combined
    # scales (amax/127/rowsum) -> dequant+normalize fused per shard, no
    # transpose, overlapped with the bandwidth-bound fetch.
    out_full = np.empty((H, N_NODES, F), np.float32)
    aggq = out_by_name["aggq"]
    sclo = out_by_name["sclo"]
    wn = w[:, 0, :]  # [H, F]

    def _fetch_and_norm(c):
        shard = np.asarray(aggq.addressable_shards[c].data)  # [2,98,128,128]
        scl = np.asarray(sclo.addressable_shards[c].data)    # [98,128,2]
        q, rr = divmod(c, 2)
        hs = slice(2 * rr, 2 * rr + 2)
        lo = q * NODES_Q
        nn = min(NODES_Q, N_NODES - lo)
        if nn <= 0:
            return
        fac = scl.reshape(NODES_Q, 2).T[:, :nn].astype(np.float32)
        dst_v = out_full[hs, lo:lo + nn, :]
        np.multiply(shard.reshape(2, NODES_Q, F)[:, :nn],
                    fac[:, :, None], out=dst_v, casting="unsafe")
        dst_v *= wn[hs][:, None, :]

    list(pool.map(_fetch_and_norm, range(NCORES)))
    runner.next_zeros = runner.zeros()  # pre-dispatch for the next call
    pool.shutdown(wait=False)
    return out_full
